# revision 2
# baseline (speedup 1.0000x reference)
"""Trainium2 Bass kernel: 3D interpolation (2x bilinear in H,W + 2x nearest in D).

Input  x: (2, 1, 128, 128, 128) f32
Output  : (2, 1, 256, 256, 256) f32

Math (scale=2, align_corners=False): separable 2-tap filter {0.75, 0.25}:
  out[2p]   = 0.25*x[p-1] + 0.75*x[p]   (clamped at p=0)
  out[2p+1] = 0.75*x[p]   + 0.25*x[p+1] (clamped at p=H-1)
applied along H then W; the D axis is a pure repeat (each plane written twice).

Sharding: pure data-parallel over the 256 (b, d) slices -> 32 slices/core on
8 cores; no communication.

Per-core pipeline (S slices per iteration):
  - load x tile (p=h, s, w) via one DMA
  - build partition-shifted copies xup/xdn via SBUF->SBUF DMAs (+1-partition
    clamp DMAs); clamps make the boundary formulas exact automatically
  - H-stage: E = 0.25*xdn + 0.75*x, O = 0.25*xup + 0.75*x (stt on DVE, 0.75*x
    precomputed on ACT)
  - W-stage on each of E/O: interior via two stt ops with free-axis-shifted
    operands and stride-2 writes; boundary cols via tiny copies
  - store: 4 big DMAs per iteration (E/O parity x 2 D-repeats), strided rows
"""
import numpy as np

N_CORES = 8
B, D, H, W = 2, 128, 128, 128
SLICES_PER_CORE = (B * D) // N_CORES  # 32
S = 8                                 # slices per pipeline iteration
N_ITERS = SLICES_PER_CORE // S
BUFS = 3

_cache = {}


def _build():
    from concourse import bacc, mybir
    from concourse.tile import TileContext

    F32 = mybir.dt.float32
    Copy = mybir.ActivationFunctionType.Copy
    mult, add = mybir.AluOpType.mult, mybir.AluOpType.add

    nc = bacc.Bacc("TRN2", target_bir_lowering=False, debug=False)
    x_ext = nc.declare_dram_parameter(
        "x", [SLICES_PER_CORE, H, W], F32, isOutput=False)
    y_ext = nc.declare_dram_parameter(
        "y", [2 * SLICES_PER_CORE, 2 * H, 2 * W], F32, isOutput=True)

    with TileContext(nc) as tc:
        with tc.tile_pool(name="pool", bufs=BUFS) as pool:
            for i in range(N_ITERS):
                xt = pool.tile([H, S, W], F32)
                xup = pool.tile([H, S, W], F32)
                xdn = pool.tile([H, S, W], F32)
                t3 = pool.tile([H, S, W], F32)
                E = pool.tile([H, S, W], F32)
                O = pool.tile([H, S, W], F32)
                u3e = pool.tile([H, S, W], F32)
                u3o = pool.tile([H, S, W], F32)
                XE = pool.tile([H, S, 2 * W], F32)
                XO = pool.tile([H, S, 2 * W], F32)

                # load: DRAM (s, h, w) iterated as (h, s, w) to match SBUF
                nc.sync.dma_start(
                    out=xt[:],
                    in_=x_ext[i * S:(i + 1) * S].rearrange("s p w -> p s w"))

                # partition shifts with boundary clamps
                nc.sync.dma_start(out=xup[0:H - 1], in_=xt[1:H])
                nc.sync.dma_start(out=xup[H - 1:H], in_=xt[H - 1:H])
                nc.sync.dma_start(out=xdn[1:H], in_=xt[0:H - 1])
                nc.sync.dma_start(out=xdn[0:1], in_=xt[0:1])

                # H-stage
                nc.scalar.activation(t3[:], xt[:], Copy, scale=0.75)
                nc.vector.scalar_tensor_tensor(
                    out=E[:], in0=xdn[:], scalar=0.25, in1=t3[:],
                    op0=mult, op1=add)
                nc.vector.scalar_tensor_tensor(
                    out=O[:], in0=xup[:], scalar=0.25, in1=t3[:],
                    op0=mult, op1=add)

                # W-stage: T (H, S, W) -> X (H, S, 2W)
                for T, u3, X in ((E, u3e, XE), (O, u3o, XO)):
                    nc.scalar.activation(u3[:], T[:], Copy, scale=0.75)
                    # odd cols 2j+1 = 0.25*T[j+1] + 0.75*T[j], j=0..W-2
                    nc.vector.scalar_tensor_tensor(
                        out=X[:, :, 1:2 * W - 1:2], in0=T[:, :, 1:W],
                        scalar=0.25, in1=u3[:, :, 0:W - 1], op0=mult, op1=add)
                    # even cols 2j = 0.25*T[j-1] + 0.75*T[j], j=1..W-1
                    nc.vector.scalar_tensor_tensor(
                        out=X[:, :, 2:2 * W:2], in0=T[:, :, 0:W - 1],
                        scalar=0.25, in1=u3[:, :, 1:W], op0=mult, op1=add)
                    # boundary cols (clamp)
                    nc.scalar.activation(
                        X[:, :, 0:1], T[:, :, 0:1], Copy)
                    nc.scalar.activation(
                        X[:, :, 2 * W - 1:2 * W], T[:, :, W - 1:W], Copy)

                # store: parity tiles x 2 D-repeats
                for r in range(2):
                    base = 2 * i * S + r
                    nc.sync.dma_start(
                        out=y_ext[base:base + 2 * S - 1:2, 0:2 * H:2, :]
                        .rearrange("s p w -> p s w"),
                        in_=XE[:])
                    nc.sync.dma_start(
                        out=y_ext[base:base + 2 * S - 1:2, 1:2 * H:2, :]
                        .rearrange("s p w -> p s w"),
                        in_=XO[:])

    nc.finalize()
    return nc


def _get_nc():
    if "nc" not in _cache:
        _cache["nc"] = _build()
    return _cache["nc"]


def _run(x, trace=False, **kw):
    from concourse.bass_utils import run_bass_kernel_spmd

    nc = _get_nc()
    x = np.asarray(x, dtype=np.float32)
    xr = x.reshape(B * D, H, W)
    in_maps = [
        {"x": np.ascontiguousarray(
            xr[k * SLICES_PER_CORE:(k + 1) * SLICES_PER_CORE])}
        for k in range(N_CORES)
    ]
    bkr = run_bass_kernel_spmd(nc, in_maps, list(range(N_CORES)),
                               trace=trace, **kw)
    out = np.empty((B, 2 * D, 2 * H, 2 * W), dtype=np.float32)
    for k in range(N_CORES):
        g = k * SLICES_PER_CORE
        b, d0 = g // D, g % D
        out[b, 2 * d0:2 * d0 + 2 * SLICES_PER_CORE] = bkr.results[k]["y"]
    return out.reshape(B, 1, 2 * D, 2 * H, 2 * W), bkr


def kernel(x):
    return _run(x)[0]
